# revision 9
# baseline (speedup 1.0000x reference)
"""Trainium2 Bass kernel for nn_DCTFeatureModel.

Math: the reference pipeline (3D DCT-II over [time-in-bin, H, W], mean over
DCT bins, full-receptive-field Conv3d, bias, LeakyReLU) is linear up to the
LeakyReLU, so everything folds into a single small matmul:

    feat[b,s,o] = LeakyReLU( sum_{c,t,i,j} x[b,s,c,t,i,j] * Weff[s,o,t,i,j]
                             + bias[s,o] )
    Weff[s,o,t,i,j] = (1/8) * sum_{f,p,q} Ct[f,t] Cs[p,i] Cs[q,j] W[s,o,f,p,q]

Weff is tiny (2*64*2048 floats) and computed on host. The device kernel is
memory-bound: stream x (bf16, 8.4 MB/core), reduce over the 8 DCT bins (c),
then a [128b x 2048k] @ [2048k x 64o] matmul per subwindow.

Device dataflow (per core): host casts x to bf16 in blocks
[kin=128, (c, chin, b)] per (s, g) chunk-group; each block streams as two
1 MB half-DMAs (c0-3 / c4-7) on the sync HWDGE ring so halves land in
strict order. The c-reduction is a pairwise binary tree of unit-stride
bf16 tensor_adds on DVE (2x packed mode, ~0.6 us per [128,1024] add),
pipelined against the DMA stream; the half-split lets 3 of the 7 adds per
block run while the second half is still in flight. Matmuls are
w-stationary: lhsT = Weff chunk [128k, 64o], rhs = z chunk [128k, 128b],
PSUM [64o, 128b] per subwindow. Bias + LeakyReLU fuse into ONE scalar
engine activation (Lrelu, per-partition bias) straight out of PSUM, and
the output lands as [s*64+o, b] which the host transposes for free.

Sharding: pure data-parallel over batch, 1024/8 = 128 rows per core.
"""

from contextlib import ExitStack

import ml_dtypes
import numpy as np

import concourse.bacc as bacc
import concourse.tile as tile
from concourse import mybir
from concourse.bass_utils import run_bass_kernel_spmd

# Problem shapes (hardcoded per contract)
B = 1024
NCORES = 8
BS = B // NCORES          # 128 batch rows per core
NSW = 2                   # subwindows
NBINS = 8                 # DCT bins (mean-reduced)
NDCT = 32                 # time points per bin
HW = 8
NF = 64                   # conv output filters per subwindow
K = NDCT * HW * HW        # 2048 contraction elements per (s, c)
P = 128                   # partitions
NCHUNK = K // P           # 16 k-chunks of 128
NG = 2                    # chunk-groups per s
CPG = NCHUNK // NG        # 8 chunks per group
GW = CPG * P              # 1024 columns per (c) slice of a group
NBLK = NSW * NG           # 4 (s, g) blocks
NHALF = 2 * NBLK          # 8 half-block DMAs
HCOLS = (NBINS // 2) * GW  # 4096 columns per half (4 c-slices)
OUT_F = NSW * NF          # 128 output features
SLOPE = 0.02

F32 = mybir.dt.float32
BF16 = mybir.dt.bfloat16
NP_BF16 = ml_dtypes.bfloat16

_cached = None
last_results = None


def _dct2(N):
    n = np.arange(N, dtype=np.float64)
    k = np.arange(N, dtype=np.float64)
    return 2.0 * np.cos(np.pi * (2.0 * n[None, :] + 1.0) * k[:, None] / (2.0 * N))


def _kernel_body(tc, x, w, bias, out):
    """x: [NHALF, 128, HCOLS] bf16 — half h of block (s,g) at [2*(s*NG+g)+h],
    cols (c_local, chin, b). w: [P, NSW*NCHUNK*NF] bf16 (lhsT chunks).
    bias: [OUT_F, 1] f32 (partition-major (s,o)). out: [OUT_F, BS] f32."""
    nc = tc.nc
    with ExitStack() as ctx:
        const_pool = ctx.enter_context(tc.tile_pool(name="const", bufs=1))
        xpool = ctx.enter_context(tc.tile_pool(name="xp", bufs=NHALF))
        upool = ctx.enter_context(tc.tile_pool(name="up", bufs=6))
        zpool = ctx.enter_context(tc.tile_pool(name="zp", bufs=2))
        opool = ctx.enter_context(tc.tile_pool(name="op", bufs=1))
        pft_pool = ctx.enter_context(tc.tile_pool(name="pft", bufs=1, space="PSUM"))

        # consts lead the sync ring (w is needed before the first matmul);
        # keeping the scalar engine DMA-free at startup matters: its
        # ACT_TABLE_LOAD (~1.3 us) would otherwise delay the x stream start
        w_sb = const_pool.tile([P, NSW * NCHUNK * NF], BF16)
        nc.sync.dma_start(out=w_sb, in_=w)
        bias_sb = const_pool.tile([OUT_F, 1], F32)
        nc.sync.dma_start(out=bias_sb, in_=bias)
        # Prelu slope as per-partition AP: the device Lrelu hardwires 0.01
        # and ignores `alpha`; Prelu with an alpha AP applies it exactly
        alpha_sb = const_pool.tile([OUT_F, 1], F32)
        nc.gpsimd.memset(alpha_sb, SLOPE)

        out_sb = opool.tile([OUT_F, BS], F32)
        psum_feat = [
            pft_pool.tile([NF, BS], F32, tag=f"feat{s}", name=f"psum_feat{s}")
            for s in range(NSW)
        ]

        # blocks 0-2: symmetric 1 MB halves (c0-3 | c4-7). Last block: c0-5 in
        # the first DMA, c6-7 in the second, so only one lvl0 add remains on
        # the post-stream critical path.
        splits = [6 * GW if blk == NBLK - 1 else 4 * GW for blk in range(NBLK)]
        xtiles = []
        for blk in range(NBLK):
            cut = splits[blk]
            ta = xpool.tile([P, cut], BF16, tag="xa", name=f"xa{blk}")
            nc.sync.dma_start(out=ta, in_=x[blk][:, 0:cut])
            tb = xpool.tile([P, NBINS * GW - cut], BF16, tag="xb", name=f"xb{blk}")
            nc.sync.dma_start(out=tb, in_=x[blk][:, cut:])
            xtiles.append((ta, tb))

        for blk in range(NBLK):
            s, g = divmod(blk, NG)
            h0, h1 = xtiles[blk]
            last = blk == NBLK - 1
            # pairwise c-tree, unit-stride bf16 adds (DVE 2x packed mode);
            # first-DMA partials overlap the second DMA
            u01 = upool.tile([P, GW], BF16, tag="u", name=f"u01_{blk}")
            nc.vector.tensor_add(out=u01, in0=h0[:, 0:GW], in1=h0[:, GW:2 * GW])
            u23 = upool.tile([P, GW], BF16, tag="u", name=f"u23_{blk}")
            nc.vector.tensor_add(out=u23, in0=h0[:, 2 * GW:3 * GW], in1=h0[:, 3 * GW:4 * GW])
            v0 = upool.tile([P, GW], BF16, tag="u", name=f"v0_{blk}")
            nc.vector.tensor_add(out=v0, in0=u01, in1=u23)
            u45 = upool.tile([P, GW], BF16, tag="u", name=f"u45_{blk}")
            if last:
                # c4, c5 arrived with the first DMA; fold them in pre-stream-end
                nc.vector.tensor_add(out=u45, in0=h0[:, 4 * GW:5 * GW], in1=h0[:, 5 * GW:6 * GW])
                v0b = upool.tile([P, GW], BF16, tag="u", name=f"v0b_{blk}")
                nc.vector.tensor_add(out=v0b, in0=v0, in1=u45)
                u67 = upool.tile([P, GW], BF16, tag="u", name=f"u67_{blk}")
                nc.vector.tensor_add(out=u67, in0=h1[:, 0:GW], in1=h1[:, GW:2 * GW])
                va, vb = v0b, u67
            else:
                nc.vector.tensor_add(out=u45, in0=h1[:, 0:GW], in1=h1[:, GW:2 * GW])
                u67 = upool.tile([P, GW], BF16, tag="u", name=f"u67_{blk}")
                nc.vector.tensor_add(out=u67, in0=h1[:, 2 * GW:3 * GW], in1=h1[:, 3 * GW:4 * GW])
                v1 = upool.tile([P, GW], BF16, tag="u", name=f"v1_{blk}")
                nc.vector.tensor_add(out=v1, in0=u45, in1=u67)
                va, vb = v0, v1
            z = zpool.tile([P, GW], BF16, tag="z", name=f"z_{blk}")
            # final add col-split so the first 4 matmuls overlap the second add
            nhalfz = 2 if last else 1
            step = GW // nhalfz
            for zh in range(nhalfz):
                nc.vector.tensor_add(
                    out=z[:, zh * step:(zh + 1) * step],
                    in0=va[:, zh * step:(zh + 1) * step],
                    in1=vb[:, zh * step:(zh + 1) * step],
                )
                # w-stationary matmuls: psum[o, b] += w_chunk.T @ z_chunk
                for j in range(step // P):
                    chin = zh * (step // P) + j
                    ch = g * CPG + chin
                    nc.tensor.matmul(
                        psum_feat[s],
                        lhsT=w_sb[:, (s * NCHUNK + ch) * NF:(s * NCHUNK + ch + 1) * NF],
                        rhs=z[:, chin * P:(chin + 1) * P],
                        start=(ch == 0),
                        stop=(ch == NCHUNK - 1),
                    )
            if g == NG - 1:
                # bias + LeakyReLU in one ACT instruction: Prelu(psum + bias)
                nc.scalar.activation(
                    out_sb[s * NF:(s + 1) * NF, :],
                    psum_feat[s],
                    mybir.ActivationFunctionType.Prelu,
                    bias=bias_sb[s * NF:(s + 1) * NF, :],
                    alpha=alpha_sb[s * NF:(s + 1) * NF, :],
                )
                nc.sync.dma_start(
                    out=out[s * NF:(s + 1) * NF, :],
                    in_=out_sb[s * NF:(s + 1) * NF, :],
                )


def _build():
    global _cached
    if _cached is not None:
        return _cached
    nc = bacc.Bacc(
        "TRN2",
        target_bir_lowering=False,
        debug=False,
        enable_asserts=False,
        num_devices=NCORES,
    )
    x_ap = nc.dram_tensor("x", [NBLK, P, NBINS * GW], BF16, kind="ExternalInput").ap()
    w_ap = nc.dram_tensor("w", [P, NSW * NCHUNK * NF], BF16, kind="ExternalInput").ap()
    b_ap = nc.dram_tensor("bias", [OUT_F, 1], F32, kind="ExternalInput").ap()
    out_ap = nc.dram_tensor("out", [OUT_F, BS], F32, kind="ExternalOutput").ap()
    with tile.TileContext(nc, trace_sim=False) as tc:
        _kernel_body(tc, x_ap, w_ap, b_ap, out_ap)
    nc.compile()
    _cached = nc
    return nc


def kernel(x, W, b):
    global last_results
    assert x.shape == (B, 1, NSW * NBINS * NDCT, HW, HW), x.shape
    nc = _build()

    # Host-side folding of the DCT matrices into the conv weights (tiny).
    Ct = _dct2(NDCT)                       # [f, t]
    Cs = _dct2(HW)                         # [p, i]
    Weff = np.einsum(
        "ft,pi,qj,sofpq->sotij", Ct, Cs, Cs, W.astype(np.float64), optimize=True
    ) / float(NBINS)
    Weff_k = Weff.reshape(NSW, NF, K)      # [s, o, k]
    # lhsT chunk layout: w[p, (s*NCHUNK+ch)*NF + o] = Weff_k[s, o, ch*128 + p]
    w_dev = np.ascontiguousarray(
        Weff_k.reshape(NSW, NF, NCHUNK, P).transpose(3, 0, 2, 1).reshape(P, NSW * NCHUNK * NF)
    ).astype(NP_BF16)
    bias_dev = np.ascontiguousarray(b.reshape(OUT_F, 1)).astype(np.float32)

    x2 = x.reshape(B, NSW, NBINS, NG, CPG, P)  # (b, s, c, g, chin, kin)
    in_maps = []
    for i in range(NCORES):
        xs = x2[i * BS:(i + 1) * BS]
        # -> [s, g, kin, c, chin, b]: contiguous [128, 4096] bf16 half-blocks
        xt = np.ascontiguousarray(xs.transpose(1, 3, 5, 2, 4, 0)).astype(NP_BF16)
        in_maps.append({
            "x": xt.reshape(NBLK, P, NBINS * GW),
            "w": w_dev,
            "bias": bias_dev,
        })
    res = run_bass_kernel_spmd(nc, in_maps, core_ids=list(range(NCORES)))
    last_results = res
    # device emits [s*64+o, b] per core; transpose back to [b, s*64+o]
    return np.concatenate([r["out"].T for r in res.results], axis=0)


# revision 11
# speedup vs baseline: 1.0414x; 1.0414x over previous
"""Trainium2 Bass kernel for nn_DCTFeatureModel.

Math: the reference pipeline (3D DCT-II over [time-in-bin, H, W], mean over
DCT bins, full-receptive-field Conv3d, bias, LeakyReLU) is linear up to the
LeakyReLU, so everything folds into a single small matmul:

    feat[b,s,o] = LeakyReLU( sum_{c,t,i,j} x[b,s,c,t,i,j] * Weff[s,o,t,i,j]
                             + bias[s,o] )
    Weff[s,o,t,i,j] = (1/8) * sum_{f,p,q} Ct[f,t] Cs[p,i] Cs[q,j] W[s,o,f,p,q]

Weff is tiny and computed on host. The device kernel is memory-bound:
stream x (bf16, 8.4 MB/core at the ~352 GB/s HBM fair share), reduce over
the 8 DCT bins (c) with a pairwise tree of unit-stride bf16 DVE adds
(2x packed mode), then w-stationary matmuls into PSUM[o, b] and a single
fused bias+LeakyReLU (Prelu with per-partition alpha — the HW Lrelu
hardwires slope 0.01) on the scalar engine per subwindow.

Tail engineering: the contraction for s1 is split unevenly into 12+4
chunk groups. The big (3 MB) s1 block streams FIRST and the small (1 MB)
s1 block LAST, so after the final byte lands only a half-width tree
(3 short adds), a col-split final add, 4 matmuls and the Prelu remain.
All DMAs are contiguous column slices of one [128, 32768] bf16 tensor on
the sync HWDGE ring (strict FIFO = deterministic landing order); w/bias
lead the ring so the scalar engine's ACT_TABLE_LOAD stays off the
stream-start path.

Sharding: pure data-parallel over batch, 1024/8 = 128 rows per core.
"""

from contextlib import ExitStack

import ml_dtypes
import numpy as np

import concourse.bacc as bacc
import concourse.tile as tile
from concourse import mybir
from concourse.bass_utils import run_bass_kernel_spmd

# Problem shapes (hardcoded per contract)
B = 1024
NCORES = 8
BS = B // NCORES          # 128 batch rows per core
NSW = 2                   # subwindows
NBINS = 8                 # DCT bins (mean-reduced)
NDCT = 32                 # time points per bin
HW = 8
NF = 64                   # conv output filters per subwindow
K = NDCT * HW * HW        # 2048 contraction elements per (s, c)
P = 128                   # partitions
NCHUNK = K // P           # 16 k-chunks of 128 per subwindow
OUT_F = NSW * NF          # 128 output features
SLOPE = 0.02
TOTCOLS = NSW * NBINS * NCHUNK * P * BS // P  # 32768 bf16 cols per partition

# blocks in STREAM order: (s, chunk_lo, chunk_hi); the 3 MB s1 block first,
# the 1 MB s1 block last
BLOCKS = [(1, 0, 12), (0, 0, 8), (0, 8, 16), (1, 12, 16)]

F32 = mybir.dt.float32
BF16 = mybir.dt.bfloat16
NP_BF16 = ml_dtypes.bfloat16

_cached = None
last_results = None


def _dct2(N):
    n = np.arange(N, dtype=np.float64)
    k = np.arange(N, dtype=np.float64)
    return 2.0 * np.cos(np.pi * (2.0 * n[None, :] + 1.0) * k[:, None] / (2.0 * N))


def _kernel_body(tc, x, w, bias, out):
    """x: [P, TOTCOLS] bf16, column-concatenated half-blocks in stream order;
    each block (s, lo, hi) is laid [kin, (c, chin in lo:hi, b)].
    w: [P, NSW*NCHUNK*NF] bf16 lhsT chunks. bias: [OUT_F, 1] f32 (s,o)-major.
    out: [OUT_F, BS] f32."""
    nc = tc.nc
    with ExitStack() as ctx:
        const_pool = ctx.enter_context(tc.tile_pool(name="const", bufs=1))
        xpool = ctx.enter_context(tc.tile_pool(name="xp", bufs=1))
        upool = ctx.enter_context(tc.tile_pool(name="up", bufs=1))
        zpool = ctx.enter_context(tc.tile_pool(name="zp", bufs=1))
        pft_pool = ctx.enter_context(tc.tile_pool(name="pft", bufs=1, space="PSUM"))

        # consts lead the sync ring: w is needed before the first matmul, and
        # issuing it here keeps the scalar engine's ACT_TABLE_LOAD (~1.3 us)
        # off the stream-start critical path
        w_sb = const_pool.tile([P, NSW * NCHUNK * NF], BF16)
        nc.sync.dma_start(out=w_sb, in_=w)
        bias_sb = const_pool.tile([OUT_F, 1], F32)
        nc.sync.dma_start(out=bias_sb, in_=bias)
        # Prelu slope as per-partition AP (HW Lrelu ignores alpha)
        alpha_sb = const_pool.tile([OUT_F, 1], F32)
        nc.gpsimd.memset(alpha_sb, SLOPE)

        out_sb = const_pool.tile([OUT_F, BS], F32)
        psum_feat = [
            pft_pool.tile([NF, BS], F32, tag=f"feat{s}", name=f"psum_feat{s}")
            for s in range(NSW)
        ]

        # stream the half-blocks (c0-3 | c4-7 of each block), strict FIFO
        halves = []
        off = 0
        for bi, (s, lo, hi) in enumerate(BLOCKS):
            gw = (hi - lo) * P            # z width for this block
            hw_cols = 4 * gw              # half-block width (4 c-slices)
            pair = []
            for h in range(2):
                t = xpool.tile([P, hw_cols], BF16, tag=f"x{bi}{h}", name=f"x{bi}{h}")
                nc.sync.dma_start(out=t, in_=x[:, off:off + hw_cols])
                off += hw_cols
                pair.append(t)
            halves.append(pair)

        for bi, (s, lo, hi) in enumerate(BLOCKS):
            gw = (hi - lo) * P
            h0, h1 = halves[bi]
            # pairwise c-tree, unit-stride bf16 adds (DVE 2x packed mode);
            # first-half partials overlap the second half's DMA
            u01 = upool.tile([P, gw], BF16, tag=f"u01_{bi}", name=f"u01_{bi}")
            # one gpsimd add on the earliest, slack-rich block as a rate probe
            eng = nc.gpsimd if bi == 0 else nc.vector
            eng.tensor_add(out=u01, in0=h0[:, 0:gw], in1=h0[:, gw:2 * gw])
            u23 = upool.tile([P, gw], BF16, tag=f"u23_{bi}", name=f"u23_{bi}")
            nc.vector.tensor_add(out=u23, in0=h0[:, 2 * gw:3 * gw], in1=h0[:, 3 * gw:4 * gw])
            v0 = upool.tile([P, gw], BF16, tag=f"v0_{bi}", name=f"v0_{bi}")
            nc.vector.tensor_add(out=v0, in0=u01, in1=u23)
            u45 = upool.tile([P, gw], BF16, tag=f"u45_{bi}", name=f"u45_{bi}")
            nc.vector.tensor_add(out=u45, in0=h1[:, 0:gw], in1=h1[:, gw:2 * gw])
            u67 = upool.tile([P, gw], BF16, tag=f"u67_{bi}", name=f"u67_{bi}")
            nc.vector.tensor_add(out=u67, in0=h1[:, 2 * gw:3 * gw], in1=h1[:, 3 * gw:4 * gw])
            v1 = upool.tile([P, gw], BF16, tag=f"v1_{bi}", name=f"v1_{bi}")
            nc.vector.tensor_add(out=v1, in0=u45, in1=u67)
            z = zpool.tile([P, gw], BF16, tag=f"z_{bi}", name=f"z_{bi}")
            # final add col-split: the first half's matmuls overlap the second
            step = gw // 2
            for zh in range(2):
                nc.vector.tensor_add(
                    out=z[:, zh * step:(zh + 1) * step],
                    in0=v0[:, zh * step:(zh + 1) * step],
                    in1=v1[:, zh * step:(zh + 1) * step],
                )
                # w-stationary matmuls: psum[o, b] += w_chunk.T @ z_chunk
                for j in range(step // P):
                    chin = zh * (step // P) + j
                    ch = lo + chin
                    nc.tensor.matmul(
                        psum_feat[s],
                        lhsT=w_sb[:, (s * NCHUNK + ch) * NF:(s * NCHUNK + ch + 1) * NF],
                        rhs=z[:, chin * P:(chin + 1) * P],
                        start=(ch == 0),
                        stop=(ch == NCHUNK - 1),
                    )
            if hi == NCHUNK:
                # bias + LeakyReLU in one ACT instruction: Prelu(psum + bias)
                nc.scalar.activation(
                    out_sb[s * NF:(s + 1) * NF, :],
                    psum_feat[s],
                    mybir.ActivationFunctionType.Prelu,
                    bias=bias_sb[s * NF:(s + 1) * NF, :],
                    alpha=alpha_sb[s * NF:(s + 1) * NF, :],
                )
                nc.sync.dma_start(
                    out=out[s * NF:(s + 1) * NF, :],
                    in_=out_sb[s * NF:(s + 1) * NF, :],
                )


def _build():
    global _cached
    if _cached is not None:
        return _cached
    nc = bacc.Bacc(
        "TRN2",
        target_bir_lowering=False,
        debug=False,
        enable_asserts=False,
        num_devices=NCORES,
    )
    x_ap = nc.dram_tensor("x", [P, TOTCOLS], BF16, kind="ExternalInput").ap()
    w_ap = nc.dram_tensor("w", [P, NSW * NCHUNK * NF], BF16, kind="ExternalInput").ap()
    b_ap = nc.dram_tensor("bias", [OUT_F, 1], F32, kind="ExternalInput").ap()
    out_ap = nc.dram_tensor("out", [OUT_F, BS], F32, kind="ExternalOutput").ap()
    with tile.TileContext(nc, trace_sim=False) as tc:
        _kernel_body(tc, x_ap, w_ap, b_ap, out_ap)
    nc.compile()
    _cached = nc
    return nc


def kernel(x, W, b):
    global last_results
    assert x.shape == (B, 1, NSW * NBINS * NDCT, HW, HW), x.shape
    nc = _build()

    # Host-side folding of the DCT matrices into the conv weights (tiny).
    Ct = _dct2(NDCT)                       # [f, t]
    Cs = _dct2(HW)                         # [p, i]
    Weff = np.einsum(
        "ft,pi,qj,sofpq->sotij", Ct, Cs, Cs, W.astype(np.float64), optimize=True
    ) / float(NBINS)
    Weff_k = Weff.reshape(NSW, NF, K)      # [s, o, k]
    # lhsT chunk layout: w[p, (s*NCHUNK+ch)*NF + o] = Weff_k[s, o, ch*128 + p]
    w_dev = np.ascontiguousarray(
        Weff_k.reshape(NSW, NF, NCHUNK, P).transpose(3, 0, 2, 1).reshape(P, NSW * NCHUNK * NF)
    ).astype(NP_BF16)
    bias_dev = np.ascontiguousarray(b.reshape(OUT_F, 1)).astype(np.float32)

    x2 = x.reshape(B, NSW, NBINS, NCHUNK, P)  # (b, s, c, ch, kin)
    in_maps = []
    for i in range(NCORES):
        xs = x2[i * BS:(i + 1) * BS]
        cols = []
        for s, lo, hi in BLOCKS:
            t = xs[:, s, :, lo:hi, :].transpose(3, 1, 2, 0)  # [kin, c, chin, b]
            t = t.reshape(P, NBINS * (hi - lo) * BS)
            cols.append(t[:, :t.shape[1] // 2])
            cols.append(t[:, t.shape[1] // 2:])
        xt = np.ascontiguousarray(np.concatenate(cols, axis=1)).astype(NP_BF16)
        in_maps.append({"x": xt, "w": w_dev, "bias": bias_dev})
    res = run_bass_kernel_spmd(nc, in_maps, core_ids=list(range(NCORES)))
    last_results = res
    # device emits [s*64+o, b] per core; transpose back to [b, s*64+o]
    return np.concatenate([r["out"].T for r in res.results], axis=0)
